# revision 7
# baseline (speedup 1.0000x reference)
"""2-layer GCN on 8 Trainium2 NeuronCores via Bass/Tile.

Sharding: nodes row-sharded across the 8 cores (12500 each, padded to
12544); edges sharded by destination core and grouped by source shard
(the 8 GPSIMD Q7 cores).  Dense transforms run feats-on-partitions in
bf16 with the weights stationary; the 16-dim xw is all-gathered into a
per-core fp32 SBUF table [128 partitions = 8 src shards x 16 feats,
12544 nodes]; messages are gathered with gpsimd.ap_gather (the kernel's
hard bottleneck at ~27.5 ns/idx/core), weighted with a streamed bf16
expanded weight tile and segment-reduced on the vector engine
((dst,group) runs class-sorted by length with water-filled region
capacities + upward spill to cut padding), permuted back to dst order
with a second ap_gather and combined across groups with one PE matmul
contracting the partition axis.  W2 is applied after the second spmm
(it commutes with the segment sum).
"""
import sys

for _p in ("/opt/trn_rl_repo",):
    if _p not in sys.path:
        sys.path.insert(0, _p)

import numpy as np
import ml_dtypes

BF16 = ml_dtypes.bfloat16

N_CORES = 8
N_NODES = 100000
SHARD = 12500
SHARD_PAD = 12544          # 98*128
IN_DIM = 512
HID = 16
OUT = 7
CHUNK = 2048               # gather slots per ap_gather call (per group)
SLAB = 512                 # dense-phase node slab
PERMJ = 512                # perm-gather block

TRACE = False              # test harness sets True to capture an NTFF profile
LAST = {}                  # last run's BassKernelResults (for the harness)


def _region_caps(counts, max_c):
    """Water-filled per-class region capacities with upward spill.

    counts: [64, max_c+1] run counts per (core,group) per class.
    Returns caps [max_c+2] and the promoted per-cg counts.
    """
    ncg = counts.shape[0]
    adj = np.zeros((ncg, max_c + 2), dtype=np.int64)
    adj[:, :max_c + 1] = counts
    caps = np.zeros(max_c + 2, dtype=np.int64)
    for c in range(1, max_c + 2):
        col = adj[:, c]
        hi = int(col.max())
        if c <= max_c:
            tgt = int(np.ceil(col.mean() + 1.0 * col.std() + 1))
            caps[c] = min(hi, tgt)
        else:
            caps[c] = hi
        if hi > caps[c] and c <= max_c:
            over = np.maximum(col - caps[c], 0)
            adj[:, c] = col - over
            adj[:, c + 1] += over
    return caps, adj


def _preprocess(edge_src, edge_dst, edge_weight):
    core = (edge_dst // SHARD).astype(np.int64)
    grp = (edge_src // SHARD).astype(np.int64)
    dloc = (edge_dst - core * SHARD).astype(np.int64)
    sloc = (edge_src - grp * SHARD).astype(np.int32)

    key = (core * 8 + grp) * SHARD + dloc
    order = np.argsort(key, kind="stable")
    key_s = key[order]
    sloc_s = sloc[order]
    w_s = edge_weight[order].astype(np.float32)

    uk, start, cnt = np.unique(key_s, return_index=True, return_counts=True)
    run_cg = (uk // SHARD).astype(np.int64)          # core*8+grp
    run_dst = (uk % SHARD).astype(np.int64)
    run_cnt = cnt.astype(np.int64)

    max_c = int(run_cnt.max())
    counts = np.zeros((N_CORES * 8, max_c + 1), dtype=np.int64)
    np.add.at(counts, (run_cg, run_cnt), 1)
    caps, _ = _region_caps(counts, max_c)
    max_c2 = len(caps) - 1
    caps[1] += 1               # shared zero-valued dummy run (last class-1 slot)

    # slot layout: class regions ascending, runs never straddle CHUNK
    # boundaries
    regions = []               # (class, slot_start, run_start, n_runs)
    slot = 0
    run_base = 0
    class_run_off = np.zeros(max_c2 + 2, dtype=np.int64)
    for c in range(1, max_c2 + 1):
        class_run_off[c] = run_base
        n = int(caps[c])
        if n == 0:
            class_run_off[c + 1] = run_base
            continue
        done = 0
        while done < n:
            room = CHUNK - (slot % CHUNK)
            fit = min(n - done, room // c)
            if fit == 0:
                slot += room
                continue
            regions.append((c, slot, run_base + done, fit))
            slot += fit * c
            done += fit
        run_base += n
    class_run_off[max_c2 + 1] = run_base
    total_slots = ((slot + CHUNK - 1) // CHUNK) * CHUNK
    total_runs = run_base
    runs_pad = ((total_runs + 127) // 128) * 128
    assert runs_pad <= 32768 and total_slots // 16 * 16 == total_slots

    run_slot = np.zeros(max(total_runs, 1), dtype=np.int64)
    for (c, s0, r0, n) in regions:
        run_slot[r0:r0 + n] = s0 + np.arange(n, dtype=np.int64) * c
    zero_run = class_run_off[2] - 1     # reserved last class-1 run

    idx_all = np.zeros((N_CORES, 8, total_slots), dtype=np.int16)
    w_all = np.zeros((N_CORES, 8, total_slots), dtype=np.float32)
    perm_all = np.zeros((N_CORES, 8, SHARD_PAD), dtype=np.int16)

    for co in range(N_CORES):
        for g in range(8):
            cg = co * 8 + g
            sel = run_cg == cg
            rc = run_cnt[sel]
            rd = run_dst[sel]
            rstart = start[sel]
            o = np.argsort(rc, kind="stable")
            rc, rd, rstart = rc[o], rd[o], rstart[o]
            # assign runs to regions, promoting overflow upward; never
            # let a run land in the reserved dummy slot of class 1
            ridx = np.zeros(len(rc), dtype=np.int64)
            pos = 0
            used_prev = 0
            for c in range(1, max_c2 + 1):
                k = int((rc == c).sum())
                # runs entering class c: leftover promoted from below + native
                n_here = used_prev + k
                fit = min(n_here, int(caps[c]) - (1 if c == 1 else 0))
                ridx[pos:pos + fit] = class_run_off[c] + np.arange(fit)
                pos += fit
                used_prev = n_here - fit
            assert used_prev == 0, "run overflow beyond top class"
            slots = run_slot[ridx]
            ia = idx_all[co, g]
            wa = w_all[co, g]
            if len(rc):
                tot = int(rc.sum())
                within = np.arange(tot, dtype=np.int64) - np.repeat(
                    np.concatenate([[0], np.cumsum(rc)[:-1]]), rc)
                e_pos = np.repeat(rstart, rc) + within
                s_pos = np.repeat(slots, rc) + within
                ia[s_pos] = sloc_s[e_pos]
                wa[s_pos] = w_s[e_pos]
            pi = np.full(SHARD_PAD, zero_run, dtype=np.int64)
            pi[rd] = ridx
            perm_all[co, g, :] = pi.astype(np.int16)

    def wrap(a):   # [8, S] -> [128, S//16] (i -> partition 16g+i%16, col i//16)
        S = a.shape[1]
        o = np.zeros((128, S // 16), dtype=a.dtype)
        for g in range(8):
            o[16 * g:16 * g + 16, :] = a[g].reshape(-1, 16).T
        return o

    idx_w = np.stack([wrap(idx_all[co]) for co in range(N_CORES)])
    perm_w = np.stack([wrap(perm_all[co]) for co in range(N_CORES)])
    # expanded bf16 weights, chunk-contiguous: [cores, NCH, 128, CHUNK]
    nch = total_slots // CHUNK
    w16 = np.repeat(w_all, 16, axis=1).astype(BF16)          # [cores,128,S]
    w16 = w16.reshape(N_CORES, 128, nch, CHUNK).transpose(0, 2, 1, 3).copy()

    return dict(idx=idx_w, perm=perm_w, w=w16, regions=regions,
                total_slots=total_slots, runs_pad=runs_pad)


def _build(total_slots, regions, runs_pad):
    from concourse import bass, bacc, tile, mybir
    dt = mybir.dt
    AF = mybir.ActivationFunctionType
    nc = bacc.Bacc("TRN2", target_bir_lowering=False, debug=False,
                   num_devices=N_CORES)

    KB = IN_DIM // 128   # 4 k-blocks
    NSLAB = (SHARD_PAD + SLAB - 1) // SLAB
    NCH = total_slots // CHUNK

    fd = nc.dram_tensor("fT", [NSLAB, 128, KB, SLAB], dt.bfloat16, kind="ExternalInput").ap()
    md = nc.dram_tensor("mT", [NSLAB, 128, KB, SLAB], dt.bfloat16, kind="ExternalInput").ap()
    m2d = nc.dram_tensor("m2T", [HID, SHARD_PAD], dt.float32, kind="ExternalInput").ap()
    W1d = nc.dram_tensor("W1", [IN_DIM, HID], dt.bfloat16, kind="ExternalInput").ap()
    b1d = nc.dram_tensor("b1", [HID, 1], dt.float32, kind="ExternalInput").ap()
    W2d = nc.dram_tensor("W2", [HID, OUT], dt.float32, kind="ExternalInput").ap()
    b2d = nc.dram_tensor("b2", [OUT, 1], dt.float32, kind="ExternalInput").ap()
    idxd = nc.dram_tensor("idx", [128, total_slots // 16], dt.int16, kind="ExternalInput").ap()
    permd = nc.dram_tensor("perm", [128, SHARD_PAD // 16], dt.int16, kind="ExternalInput").ap()
    wd = nc.dram_tensor("w", [NCH, 128, CHUNK], dt.bfloat16, kind="ExternalInput").ap()
    seld = nc.dram_tensor("sel", [128, HID], dt.float32, kind="ExternalInput").ap()
    outd = nc.dram_tensor("out", [OUT, SHARD_PAD], dt.float32, kind="ExternalOutput").ap()

    with tile.TileContext(nc) as tc:
        with tc.tile_pool(name="const", bufs=1) as cp, \
             tc.tile_pool(name="big", bufs=1) as bp, \
             tc.tile_pool(name="ps", bufs=3, space="PSUM") as ps, \
             tc.tile_pool(name="dram", bufs=1, space="DRAM") as dp:

            W1 = cp.tile([128, KB, HID], dt.bfloat16)
            b1 = cp.tile([HID, 1], dt.float32)
            W2 = cp.tile([HID, OUT], dt.float32)
            b2 = cp.tile([OUT, 1], dt.float32)
            sel = cp.tile([128, HID], dt.float32)
            idxs = cp.tile([128, total_slots // 16], dt.int16)
            perm = cp.tile([128, SHARD_PAD // 16], dt.int16)
            nc.sync.dma_start(out=W1[:, :, :], in_=W1d.rearrange("(a b) h -> b a h", b=128))
            nc.sync.dma_start(out=b1[:], in_=b1d[:])
            nc.sync.dma_start(out=W2[:], in_=W2d[:])
            nc.sync.dma_start(out=b2[:], in_=b2d[:])
            nc.sync.dma_start(out=sel[:], in_=seld[:])
            nc.sync.dma_start(out=idxs[:], in_=idxd[:])
            nc.sync.dma_start(out=perm[:], in_=permd[:])

            table = bp.tile([128, SHARD_PAD], dt.float32, tag="table")
            st1 = bp.tile([128, runs_pad], dt.float32, tag="st1")

            gi1 = dp.tile([HID, SHARD_PAD], dt.float32, tag="gi1")
            gi2 = dp.tile([HID, SHARD_PAD], dt.float32, tag="gi2")
            go1 = dp.tile([128, SHARD_PAD], dt.float32, tag="go1", addr_space="Shared")
            go2 = dp.tile([128, SHARD_PAD], dt.float32, tag="go2", addr_space="Shared")

            # ---- dense layer 1 (bf16): gi1 = W1^T (f.m) + b1 ------------
            with tc.tile_pool(name="dense", bufs=6) as wkd:
                for s in range(NSLAB):
                    off = s * SLAB
                    jn = min(SLAB, SHARD_PAD - off)
                    f = wkd.tile([128, KB, SLAB], dt.bfloat16, tag="f")
                    m = wkd.tile([128, KB, SLAB], dt.bfloat16, tag="m")
                    nc.sync.dma_start(out=f[:, :, :jn], in_=fd[s, :, :, :jn])
                    nc.sync.dma_start(out=m[:, :, :jn], in_=md[s, :, :, :jn])
                    nc.vector.tensor_tensor(out=f[:, :, :jn], in0=f[:, :, :jn],
                                            in1=m[:, :, :jn], op=mybir.AluOpType.mult)
                    acc = ps.tile([HID, SLAB], dt.float32, tag="acc")
                    for k in range(KB):
                        nc.tensor.matmul(out=acc[:, :jn], lhsT=W1[:, k, :], rhs=f[:, k, :jn],
                                         start=(k == 0), stop=(k == KB - 1))
                    xw = wkd.tile([HID, SLAB], dt.float32, tag="xw")
                    nc.scalar.activation(out=xw[:, :jn], in_=acc[:, :jn],
                                         func=AF.Identity, bias=b1[:], scale=1.0)
                    nc.sync.dma_start(out=gi1[:, off:off + jn], in_=xw[:, :jn])

            def allgather(gi, go):
                nc.gpsimd.collective_compute(
                    "AllGather", mybir.AluOpType.bypass,
                    replica_groups=[list(range(N_CORES))],
                    ins=[gi.opt()], outs=[go.opt()])
                # split across 16 DMA queues so the 6.4 MB table load
                # doesn't serialize on one engine
                tw = SHARD_PAD // 16
                for q in range(16):
                    nc.sync.dma_start(out=table[:, q * tw:(q + 1) * tw],
                                      in_=go[:, q * tw:(q + 1) * tw])

            def spmm(wk):
                for ci in range(NCH):
                    cs = ci * CHUNK
                    wch = wk.tile([128, CHUNK], dt.bfloat16, tag="wch")
                    nc.sync.dma_start(out=wch[:, :], in_=wd[ci])
                    msg = wk.tile([128, CHUNK], dt.float32, tag="msg")
                    nc.gpsimd.ap_gather(
                        out_ap=msg[:, :], in_ap=table[:, :],
                        idxs_ap=idxs[:, cs // 16:(cs + CHUNK) // 16],
                        channels=128, num_elems=SHARD_PAD, d=1, num_idxs=CHUNK)
                    nc.vector.tensor_tensor(out=msg[:, :], in0=msg[:, :],
                                            in1=wch[:, :], op=mybir.AluOpType.mult)
                    for (c, s0, r0, n) in regions:
                        if s0 < cs or s0 >= cs + CHUNK:
                            continue
                        if c == 1:
                            nc.vector.tensor_copy(out=st1[:, r0:r0 + n],
                                                  in_=msg[:, s0 - cs:s0 - cs + n])
                        else:
                            v = msg[:, s0 - cs:s0 - cs + n * c].rearrange(
                                "p (r c) -> p r c", c=c)
                            nc.vector.tensor_reduce(out=st1[:, r0:r0 + n], in_=v,
                                                    axis=mybir.AxisListType.X,
                                                    op=mybir.AluOpType.add)

            def permloop(wk, post):
                for j in range(0, SHARD_PAD, PERMJ):
                    jn = min(PERMJ, SHARD_PAD - j)
                    al = wk.tile([128, PERMJ], dt.float32, tag="al")
                    nc.gpsimd.ap_gather(
                        out_ap=al[:, :jn], in_ap=st1[:, :],
                        idxs_ap=perm[:, j // 16:(j + jn) // 16],
                        channels=128, num_elems=runs_pad, d=1, num_idxs=jn)
                    acc2 = ps.tile([HID, PERMJ], dt.float32, tag="acc")
                    nc.tensor.matmul(out=acc2[:, :jn], lhsT=sel[:, :], rhs=al[:, :jn],
                                     start=True, stop=True)
                    post(wk, j, jn, acc2)

            # ---- spmm 1 + relu + mask2, allgather 2 ---------------------
            allgather(gi1, go1)
            with tc.tile_pool(name="spmm1", bufs=4) as wk:
                spmm(wk)

                def post1(wk, j, jn, acc2):
                    xw1 = wk.tile([HID, PERMJ], dt.float32, tag="xw1")
                    nc.scalar.activation(out=xw1[:, :jn], in_=acc2[:, :jn],
                                         func=AF.Relu, scale=1.0)
                    m2c = wk.tile([HID, PERMJ], dt.float32, tag="m2c")
                    nc.sync.dma_start(out=m2c[:, :jn], in_=m2d[:, j:j + jn])
                    nc.vector.tensor_tensor(out=xw1[:, :jn], in0=xw1[:, :jn],
                                            in1=m2c[:, :jn], op=mybir.AluOpType.mult)
                    nc.sync.dma_start(out=gi2[:, j:j + jn], in_=xw1[:, :jn])

                permloop(wk, post1)

            allgather(gi2, go2)

            # ---- spmm 2, out = W2^T h2 + b2 -----------------------------
            with tc.tile_pool(name="spmm2", bufs=4) as wk:
                spmm(wk)

                def post2(wk, j, jn, acc2):
                    h2s = wk.tile([HID, PERMJ], dt.float32, tag="h2s")
                    nc.scalar.activation(out=h2s[:, :jn], in_=acc2[:, :jn],
                                         func=AF.Identity, scale=1.0)
                    acc3 = ps.tile([OUT, PERMJ], dt.float32, tag="acc3")
                    nc.tensor.matmul(out=acc3[:, :jn], lhsT=W2[:, :], rhs=h2s[:, :jn],
                                     start=True, stop=True)
                    o = wk.tile([OUT, PERMJ], dt.float32, tag="o")
                    nc.scalar.activation(out=o[:, :jn], in_=acc3[:, :jn],
                                         func=AF.Identity, bias=b2[:], scale=1.0)
                    nc.sync.dma_start(out=outd[:, j:j + jn], in_=o[:, :jn])

                permloop(wk, post2)

    nc.compile()
    return nc


def kernel(features, edge_src, edge_dst, edge_weight, mask1, mask2,
           W1, b1, W2, b2):
    from concourse.bass_utils import run_bass_kernel_spmd

    features = np.asarray(features, dtype=np.float32)
    mask1 = np.asarray(mask1, dtype=np.float32)
    mask2 = np.asarray(mask2, dtype=np.float32)
    edge_src = np.asarray(edge_src)
    edge_dst = np.asarray(edge_dst)
    edge_weight = np.asarray(edge_weight, dtype=np.float32)

    pp = _preprocess(edge_src, edge_dst, edge_weight)
    nc = _build(pp["total_slots"], pp["regions"], pp["runs_pad"])

    sel = np.zeros((128, HID), dtype=np.float32)
    for g in range(8):
        for f in range(HID):
            sel[16 * g + f, f] = 1.0

    KB = IN_DIM // 128
    NSLAB = (SHARD_PAD + SLAB - 1) // SLAB

    def slabify(arr_t):        # [IN_DIM, SHARD_PAD] -> [NSLAB, 128, KB, SLAB]
        o = np.zeros((NSLAB, 128, KB, SLAB), dtype=BF16)
        for s in range(NSLAB):
            jn = min(SLAB, SHARD_PAD - s * SLAB)
            blk = arr_t[:, s * SLAB:s * SLAB + jn]          # [512, jn]
            o[s, :, :, :jn] = blk.reshape(KB, 128, jn).transpose(1, 0, 2)
        return o

    in_maps = []
    for c in range(N_CORES):
        lo, hi = c * SHARD, (c + 1) * SHARD
        fT = np.zeros((IN_DIM, SHARD_PAD), dtype=BF16)
        mT = np.zeros((IN_DIM, SHARD_PAD), dtype=BF16)
        m2T = np.zeros((HID, SHARD_PAD), dtype=np.float32)
        fT[:, :SHARD] = features[lo:hi].T.astype(BF16)
        mT[:, :SHARD] = mask1[lo:hi].T.astype(BF16)
        m2T[:, :SHARD] = mask2[lo:hi].T
        in_maps.append({
            "fT": slabify(fT), "mT": slabify(mT), "m2T": m2T,
            "W1": np.asarray(W1, dtype=np.float32).reshape(IN_DIM, HID).astype(BF16),
            "b1": np.asarray(b1, dtype=np.float32).reshape(HID, 1),
            "W2": np.asarray(W2, dtype=np.float32).reshape(HID, OUT),
            "b2": np.asarray(b2, dtype=np.float32).reshape(OUT, 1),
            "idx": pp["idx"][c], "perm": pp["perm"][c], "w": pp["w"][c],
            "sel": sel,
        })

    res = run_bass_kernel_spmd(nc, in_maps, core_ids=list(range(N_CORES)),
                               trace=TRACE)
    LAST["res"] = res
    out = np.zeros((N_NODES, OUT), dtype=np.float32)
    for c in range(N_CORES):
        out[c * SHARD:(c + 1) * SHARD] = res.results[c]["out"][:, :SHARD].T
    return out


# revision 11
# speedup vs baseline: 1.0272x; 1.0272x over previous
"""2-layer GCN on 8 Trainium2 NeuronCores via Bass/Tile.

Sharding: nodes row-sharded across the 8 cores (12500 each, padded to
12544); edges sharded by destination core and grouped by source shard
(the 8 GPSIMD Q7 cores).  Dense transforms run feats-on-partitions in
bf16 with the weights stationary; the 16-dim xw is all-gathered into a
per-core fp32 SBUF table [128 partitions = 8 src shards x 16 feats,
12544 nodes]; messages are gathered with gpsimd.ap_gather (the kernel's
hard bottleneck at ~27.5 ns/idx/core), weighted with a streamed bf16
expanded weight tile and segment-reduced on the vector engine
((dst,group) runs class-sorted by length with water-filled region
capacities + upward spill to cut padding), permuted back to dst order
with a second ap_gather and combined across groups with one PE matmul
contracting the partition axis.  W2 is applied after the second spmm
(it commutes with the segment sum).
"""
import sys

for _p in ("/opt/trn_rl_repo",):
    if _p not in sys.path:
        sys.path.insert(0, _p)

import numpy as np
import ml_dtypes

BF16 = ml_dtypes.bfloat16

N_CORES = 8
N_NODES = 100000
SHARD = 12500
SHARD_PAD = 12544          # 98*128
IN_DIM = 512
HID = 16
OUT = 7
CHUNK = 2048               # gather slots per ap_gather call (per group)
SLAB = 1024                # dense-phase node slab
PERMJ = 512                # perm-gather block

TRACE = False              # test harness sets True to capture an NTFF profile
LAST = {}                  # last run's BassKernelResults (for the harness)


def _region_caps(counts, max_c):
    """Water-filled per-class region capacities with upward spill.

    counts: [64, max_c+1] run counts per (core,group) per class.
    Returns caps [max_c+2] and the promoted per-cg counts.
    """
    ncg = counts.shape[0]
    adj = np.zeros((ncg, max_c + 2), dtype=np.int64)
    adj[:, :max_c + 1] = counts
    caps = np.zeros(max_c + 2, dtype=np.int64)
    for c in range(1, max_c + 2):
        col = adj[:, c]
        hi = int(col.max())
        if c <= max_c:
            tgt = int(np.ceil(col.mean() + 1.0 * col.std() + 1))
            caps[c] = min(hi, tgt)
        else:
            caps[c] = hi
        if hi > caps[c] and c <= max_c:
            over = np.maximum(col - caps[c], 0)
            adj[:, c] = col - over
            adj[:, c + 1] += over
    return caps, adj


def _preprocess(edge_src, edge_dst, edge_weight):
    core = (edge_dst // SHARD).astype(np.int64)
    grp = (edge_src // SHARD).astype(np.int64)
    dloc = (edge_dst - core * SHARD).astype(np.int64)
    sloc = (edge_src - grp * SHARD).astype(np.int32)

    key = (core * 8 + grp) * SHARD + dloc
    order = np.argsort(key, kind="stable")
    key_s = key[order]
    sloc_s = sloc[order]
    w_s = edge_weight[order].astype(np.float32)

    uk, start, cnt = np.unique(key_s, return_index=True, return_counts=True)
    run_cg = (uk // SHARD).astype(np.int64)          # core*8+grp
    run_dst = (uk % SHARD).astype(np.int64)
    run_cnt = cnt.astype(np.int64)

    max_c = int(run_cnt.max())
    counts = np.zeros((N_CORES * 8, max_c + 1), dtype=np.int64)
    np.add.at(counts, (run_cg, run_cnt), 1)
    caps, _ = _region_caps(counts, max_c)
    max_c2 = len(caps) - 1
    caps[1] += 1               # shared zero-valued dummy run (last class-1 slot)

    # slot layout: class regions ascending, runs never straddle CHUNK
    # boundaries
    regions = []               # (class, slot_start, run_start, n_runs)
    slot = 0
    run_base = 0
    class_run_off = np.zeros(max_c2 + 2, dtype=np.int64)
    for c in range(1, max_c2 + 1):
        class_run_off[c] = run_base
        n = int(caps[c])
        if n == 0:
            class_run_off[c + 1] = run_base
            continue
        done = 0
        while done < n:
            room = CHUNK - (slot % CHUNK)
            fit = min(n - done, room // c)
            if fit == 0:
                slot += room
                continue
            regions.append((c, slot, run_base + done, fit))
            slot += fit * c
            done += fit
        run_base += n
    class_run_off[max_c2 + 1] = run_base
    total_slots = ((slot + CHUNK - 1) // CHUNK) * CHUNK
    total_runs = run_base
    runs_pad = ((total_runs + 127) // 128) * 128
    assert runs_pad <= 32768 and total_slots // 16 * 16 == total_slots

    run_slot = np.zeros(max(total_runs, 1), dtype=np.int64)
    for (c, s0, r0, n) in regions:
        run_slot[r0:r0 + n] = s0 + np.arange(n, dtype=np.int64) * c
    zero_run = class_run_off[2] - 1     # reserved last class-1 run

    idx_all = np.zeros((N_CORES, 8, total_slots), dtype=np.int16)
    w_all = np.zeros((N_CORES, 8, total_slots), dtype=np.float32)
    perm_all = np.zeros((N_CORES, 8, SHARD_PAD), dtype=np.int16)

    for co in range(N_CORES):
        for g in range(8):
            cg = co * 8 + g
            sel = run_cg == cg
            rc = run_cnt[sel]
            rd = run_dst[sel]
            rstart = start[sel]
            o = np.argsort(rc, kind="stable")
            rc, rd, rstart = rc[o], rd[o], rstart[o]
            # assign runs to regions, promoting overflow upward; never
            # let a run land in the reserved dummy slot of class 1
            ridx = np.zeros(len(rc), dtype=np.int64)
            pos = 0
            used_prev = 0
            for c in range(1, max_c2 + 1):
                k = int((rc == c).sum())
                # runs entering class c: leftover promoted from below + native
                n_here = used_prev + k
                fit = min(n_here, int(caps[c]) - (1 if c == 1 else 0))
                ridx[pos:pos + fit] = class_run_off[c] + np.arange(fit)
                pos += fit
                used_prev = n_here - fit
            assert used_prev == 0, "run overflow beyond top class"
            slots = run_slot[ridx]
            ia = idx_all[co, g]
            wa = w_all[co, g]
            if len(rc):
                tot = int(rc.sum())
                within = np.arange(tot, dtype=np.int64) - np.repeat(
                    np.concatenate([[0], np.cumsum(rc)[:-1]]), rc)
                e_pos = np.repeat(rstart, rc) + within
                s_pos = np.repeat(slots, rc) + within
                ia[s_pos] = sloc_s[e_pos]
                wa[s_pos] = w_s[e_pos]
            pi = np.full(SHARD_PAD, zero_run, dtype=np.int64)
            pi[rd] = ridx
            perm_all[co, g, :] = pi.astype(np.int16)

    def wrap(a):   # [8, S] -> [128, S//16] (i -> partition 16g+i%16, col i//16)
        S = a.shape[1]
        o = np.zeros((128, S // 16), dtype=a.dtype)
        for g in range(8):
            o[16 * g:16 * g + 16, :] = a[g].reshape(-1, 16).T
        return o

    idx_w = np.stack([wrap(idx_all[co]) for co in range(N_CORES)])
    perm_w = np.stack([wrap(perm_all[co]) for co in range(N_CORES)])
    # expanded bf16 weights, chunk-contiguous: [cores, NCH, 128, CHUNK]
    nch = total_slots // CHUNK
    w16 = np.repeat(w_all, 16, axis=1).astype(BF16)          # [cores,128,S]
    w16 = w16.reshape(N_CORES, 128, nch, CHUNK).transpose(0, 2, 1, 3).copy()

    return dict(idx=idx_w, perm=perm_w, w=w16, regions=regions,
                total_slots=total_slots, runs_pad=runs_pad, raw_slots=slot)


def _build(total_slots, regions, runs_pad, raw_slots):
    from concourse import bass, bacc, tile, mybir
    dt = mybir.dt
    AF = mybir.ActivationFunctionType
    nc = bacc.Bacc("TRN2", target_bir_lowering=False, debug=False,
                   num_devices=N_CORES)

    KB = IN_DIM // 128   # 4 k-blocks
    NSLAB = (SHARD_PAD + SLAB - 1) // SLAB
    NCH = total_slots // CHUNK

    fd = nc.dram_tensor("fT", [NSLAB, 128, KB, SLAB], dt.bfloat16, kind="ExternalInput").ap()
    md = nc.dram_tensor("mT", [NSLAB, 128, KB, SLAB], dt.bfloat16, kind="ExternalInput").ap()
    m2d = nc.dram_tensor("m2T", [HID, SHARD_PAD], dt.float32, kind="ExternalInput").ap()
    W1d = nc.dram_tensor("W1", [IN_DIM, HID], dt.bfloat16, kind="ExternalInput").ap()
    b1d = nc.dram_tensor("b1", [HID, 1], dt.float32, kind="ExternalInput").ap()
    W2d = nc.dram_tensor("W2", [HID, OUT], dt.float32, kind="ExternalInput").ap()
    b2d = nc.dram_tensor("b2", [OUT, 1], dt.float32, kind="ExternalInput").ap()
    idxd = nc.dram_tensor("idx", [128, total_slots // 16], dt.int16, kind="ExternalInput").ap()
    permd = nc.dram_tensor("perm", [128, SHARD_PAD // 16], dt.int16, kind="ExternalInput").ap()
    wd = nc.dram_tensor("w", [NCH, 128, CHUNK], dt.bfloat16, kind="ExternalInput").ap()
    seld = nc.dram_tensor("sel", [128, HID], dt.float32, kind="ExternalInput").ap()
    outd = nc.dram_tensor("out", [OUT, SHARD_PAD], dt.float32, kind="ExternalOutput").ap()

    with tile.TileContext(nc) as tc:
        with tc.tile_pool(name="const", bufs=1) as cp, \
             tc.tile_pool(name="big", bufs=1) as bp, \
             tc.tile_pool(name="ps", bufs=2, space="PSUM") as ps, \
             tc.tile_pool(name="dram", bufs=1, space="DRAM") as dp:

            W1 = cp.tile([128, KB, HID], dt.bfloat16)
            b1 = cp.tile([HID, 1], dt.float32)
            W2 = cp.tile([HID, OUT], dt.float32)
            b2 = cp.tile([OUT, 1], dt.float32)
            sel = cp.tile([128, HID], dt.float32)
            idxs = cp.tile([128, total_slots // 16], dt.int16)
            perm = cp.tile([128, SHARD_PAD // 16], dt.int16)
            nc.sync.dma_start(out=W1[:, :, :], in_=W1d.rearrange("(a b) h -> b a h", b=128))
            nc.sync.dma_start(out=b1[:], in_=b1d[:])
            nc.sync.dma_start(out=W2[:], in_=W2d[:])
            nc.sync.dma_start(out=b2[:], in_=b2d[:])
            nc.sync.dma_start(out=sel[:], in_=seld[:])
            nc.sync.dma_start(out=idxs[:], in_=idxd[:])
            nc.sync.dma_start(out=perm[:], in_=permd[:])

            table = bp.tile([128, SHARD_PAD], dt.float32, tag="table")
            st1 = bp.tile([128, runs_pad], dt.float32, tag="st1")

            tableb = bp.tile([128, SHARD_PAD], dt.bfloat16, tag="tableb")
            gi1 = dp.tile([HID, SHARD_PAD], dt.bfloat16, tag="gi1")
            gi2 = dp.tile([HID, SHARD_PAD], dt.bfloat16, tag="gi2")
            go1 = dp.tile([128, SHARD_PAD], dt.bfloat16, tag="go1", addr_space="Shared")
            go2 = dp.tile([128, SHARD_PAD], dt.bfloat16, tag="go2", addr_space="Shared")

            # ---- dense layer 1 (bf16): gi1 = W1^T (f.m) + b1 ------------
            with tc.tile_pool(name="dense", bufs=3) as wkd:
                for s in range(NSLAB):
                    off = s * SLAB
                    jn = min(SLAB, SHARD_PAD - off)
                    f = wkd.tile([128, KB, SLAB], dt.bfloat16, tag="f")
                    m = wkd.tile([128, KB, SLAB], dt.bfloat16, tag="m")
                    nc.sync.dma_start(out=f[:, :, :jn], in_=fd[s, :, :, :jn])
                    nc.sync.dma_start(out=m[:, :, :jn], in_=md[s, :, :, :jn])
                    nc.vector.tensor_tensor(out=f[:, :, :jn], in0=f[:, :, :jn],
                                            in1=m[:, :, :jn], op=mybir.AluOpType.mult)
                    acc = ps.tile([HID, SLAB], dt.float32, tag="acc")
                    for h in range(0, jn, 512):
                        hn = min(512, jn - h)
                        for k in range(KB):
                            nc.tensor.matmul(out=acc[:, h:h + hn],
                                             lhsT=W1[:, k, :],
                                             rhs=f[:, k, h:h + hn],
                                             start=(k == 0), stop=(k == KB - 1))
                    xw = wkd.tile([HID, SLAB], dt.bfloat16, tag="xw")
                    nc.scalar.activation(out=xw[:, :jn], in_=acc[:, :jn],
                                         func=AF.Identity, bias=b1[:], scale=1.0)
                    nc.sync.dma_start(out=gi1[:, off:off + jn], in_=xw[:, :jn])

            def allgather(gi, go):
                nc.gpsimd.collective_compute(
                    "AllGather", mybir.AluOpType.bypass,
                    replica_groups=[list(range(N_CORES))],
                    ins=[gi.opt()], outs=[go.opt()])
                # split across 16 DMA queues so the 3.2 MB table load
                # doesn't serialize on one engine, then upconvert on DVE
                tw = SHARD_PAD // 16
                for q in range(16):
                    nc.sync.dma_start(out=tableb[:, q * tw:(q + 1) * tw],
                                      in_=go[:, q * tw:(q + 1) * tw])
                nc.vector.tensor_copy(out=table[:, :], in_=tableb[:, :])

            def spmm(wk):
                for ci in range(NCH):
                    cs = ci * CHUNK
                    ni = min(CHUNK, ((raw_slots - cs + 31) // 32) * 32)
                    wch = wk.tile([128, CHUNK], dt.bfloat16, tag="wch")
                    nc.sync.dma_start(out=wch[:, :ni], in_=wd[ci, :, :ni])
                    msg = wk.tile([128, CHUNK], dt.float32, tag="msg")
                    nc.gpsimd.ap_gather(
                        out_ap=msg[:, :ni], in_ap=table[:, :],
                        idxs_ap=idxs[:, cs // 16:cs // 16 + (ni + 15) // 16],
                        channels=128, num_elems=SHARD_PAD, d=1, num_idxs=ni)
                    nc.vector.tensor_tensor(out=msg[:, :ni], in0=msg[:, :ni],
                                            in1=wch[:, :ni], op=mybir.AluOpType.mult)
                    for (c, s0, r0, n) in regions:
                        if s0 < cs or s0 >= cs + CHUNK:
                            continue
                        if c == 1:
                            nc.vector.tensor_copy(out=st1[:, r0:r0 + n],
                                                  in_=msg[:, s0 - cs:s0 - cs + n])
                        else:
                            v = msg[:, s0 - cs:s0 - cs + n * c].rearrange(
                                "p (r c) -> p r c", c=c)
                            nc.vector.tensor_reduce(out=st1[:, r0:r0 + n], in_=v,
                                                    axis=mybir.AxisListType.X,
                                                    op=mybir.AluOpType.add)

            def permloop(wk, post):
                for j in range(0, SHARD_PAD, PERMJ):
                    jn = min(PERMJ, SHARD_PAD - j)
                    al = wk.tile([128, PERMJ], dt.float32, tag="al")
                    nc.gpsimd.ap_gather(
                        out_ap=al[:, :jn], in_ap=st1[:, :],
                        idxs_ap=perm[:, j // 16:(j + jn) // 16],
                        channels=128, num_elems=runs_pad, d=1, num_idxs=jn)
                    acc2 = ps.tile([HID, PERMJ], dt.float32, tag="acc")
                    nc.tensor.matmul(out=acc2[:, :jn], lhsT=sel[:, :], rhs=al[:, :jn],
                                     start=True, stop=True)
                    post(wk, j, jn, acc2)

            # ---- spmm 1 + relu + mask2, allgather 2 ---------------------
            allgather(gi1, go1)
            with tc.tile_pool(name="spmm1", bufs=3) as wk:
                spmm(wk)

                def post1(wk, j, jn, acc2):
                    xw1 = wk.tile([HID, PERMJ], dt.float32, tag="xw1")
                    nc.scalar.activation(out=xw1[:, :jn], in_=acc2[:, :jn],
                                         func=AF.Relu, scale=1.0)
                    m2c = wk.tile([HID, PERMJ], dt.float32, tag="m2c")
                    nc.sync.dma_start(out=m2c[:, :jn], in_=m2d[:, j:j + jn])
                    xw1b = wk.tile([HID, PERMJ], dt.bfloat16, tag="xw1b")
                    nc.vector.tensor_tensor(out=xw1b[:, :jn], in0=xw1[:, :jn],
                                            in1=m2c[:, :jn], op=mybir.AluOpType.mult)
                    nc.sync.dma_start(out=gi2[:, j:j + jn], in_=xw1b[:, :jn])

                permloop(wk, post1)

            allgather(gi2, go2)

            # ---- spmm 2, out = W2^T h2 + b2 -----------------------------
            with tc.tile_pool(name="spmm2", bufs=3) as wk:
                spmm(wk)

                def post2(wk, j, jn, acc2):
                    h2s = wk.tile([HID, PERMJ], dt.float32, tag="h2s")
                    nc.scalar.activation(out=h2s[:, :jn], in_=acc2[:, :jn],
                                         func=AF.Identity, scale=1.0)
                    acc3 = ps.tile([OUT, PERMJ], dt.float32, tag="acc3")
                    nc.tensor.matmul(out=acc3[:, :jn], lhsT=W2[:, :], rhs=h2s[:, :jn],
                                     start=True, stop=True)
                    o = wk.tile([OUT, PERMJ], dt.float32, tag="o")
                    nc.scalar.activation(out=o[:, :jn], in_=acc3[:, :jn],
                                         func=AF.Identity, bias=b2[:], scale=1.0)
                    nc.sync.dma_start(out=outd[:, j:j + jn], in_=o[:, :jn])

                permloop(wk, post2)

    nc.compile()
    return nc


def kernel(features, edge_src, edge_dst, edge_weight, mask1, mask2,
           W1, b1, W2, b2):
    from concourse.bass_utils import run_bass_kernel_spmd

    features = np.asarray(features, dtype=np.float32)
    mask1 = np.asarray(mask1, dtype=np.float32)
    mask2 = np.asarray(mask2, dtype=np.float32)
    edge_src = np.asarray(edge_src)
    edge_dst = np.asarray(edge_dst)
    edge_weight = np.asarray(edge_weight, dtype=np.float32)

    pp = _preprocess(edge_src, edge_dst, edge_weight)
    nc = _build(pp["total_slots"], pp["regions"], pp["runs_pad"], pp["raw_slots"])

    sel = np.zeros((128, HID), dtype=np.float32)
    for g in range(8):
        for f in range(HID):
            sel[16 * g + f, f] = 1.0

    KB = IN_DIM // 128
    NSLAB = (SHARD_PAD + SLAB - 1) // SLAB

    def slabify(arr_t):        # [IN_DIM, SHARD_PAD] -> [NSLAB, 128, KB, SLAB]
        o = np.zeros((NSLAB, 128, KB, SLAB), dtype=BF16)
        for s in range(NSLAB):
            jn = min(SLAB, SHARD_PAD - s * SLAB)
            blk = arr_t[:, s * SLAB:s * SLAB + jn]          # [512, jn]
            o[s, :, :, :jn] = blk.reshape(KB, 128, jn).transpose(1, 0, 2)
        return o

    in_maps = []
    for c in range(N_CORES):
        lo, hi = c * SHARD, (c + 1) * SHARD
        fT = np.zeros((IN_DIM, SHARD_PAD), dtype=BF16)
        mT = np.zeros((IN_DIM, SHARD_PAD), dtype=BF16)
        m2T = np.zeros((HID, SHARD_PAD), dtype=np.float32)
        fT[:, :SHARD] = features[lo:hi].T.astype(BF16)
        mT[:, :SHARD] = mask1[lo:hi].T.astype(BF16)
        m2T[:, :SHARD] = mask2[lo:hi].T
        in_maps.append({
            "fT": slabify(fT), "mT": slabify(mT), "m2T": m2T,
            "W1": np.asarray(W1, dtype=np.float32).reshape(IN_DIM, HID).astype(BF16),
            "b1": np.asarray(b1, dtype=np.float32).reshape(HID, 1),
            "W2": np.asarray(W2, dtype=np.float32).reshape(HID, OUT),
            "b2": np.asarray(b2, dtype=np.float32).reshape(OUT, 1),
            "idx": pp["idx"][c], "perm": pp["perm"][c], "w": pp["w"][c],
            "sel": sel,
        })

    res = run_bass_kernel_spmd(nc, in_maps, core_ids=list(range(N_CORES)),
                               trace=TRACE)
    LAST["res"] = res
    out = np.zeros((N_NODES, OUT), dtype=np.float32)
    for c in range(N_CORES):
        out[c * SHARD:(c + 1) * SHARD] = res.results[c]["out"][:, :SHARD].T
    return out


# revision 13
# speedup vs baseline: 1.0291x; 1.0019x over previous
"""2-layer GCN on 8 Trainium2 NeuronCores via Bass/Tile.

Sharding: nodes row-sharded across the 8 cores (12500 each, padded to
12544); edges sharded by destination core and grouped by source shard
(the 8 GPSIMD Q7 cores).  Dense transforms run feats-on-partitions in
bf16 with the weights stationary; the 16-dim xw is all-gathered into a
per-core fp32 SBUF table [128 partitions = 8 src shards x 16 feats,
12544 nodes]; messages are gathered with gpsimd.ap_gather (the kernel's
hard bottleneck at ~27.5 ns/idx/core), weighted with a streamed bf16
expanded weight tile and segment-reduced on the vector engine
((dst,group) runs class-sorted by length with water-filled region
capacities + upward spill to cut padding), permuted back to dst order
with a second ap_gather and combined across groups with one PE matmul
contracting the partition axis.  W2 is applied after the second spmm
(it commutes with the segment sum).
"""
import sys

for _p in ("/opt/trn_rl_repo",):
    if _p not in sys.path:
        sys.path.insert(0, _p)

import numpy as np
import ml_dtypes

BF16 = ml_dtypes.bfloat16

N_CORES = 8
N_NODES = 100000
SHARD = 12500
SHARD_PAD = 12544          # 98*128
IN_DIM = 512
HID = 16
OUT = 7
CHUNK = 2048               # gather slots per ap_gather call (per group)
SLAB = 1024                # dense-phase node slab
PERMJ = 512                # perm-gather block

TRACE = False              # test harness sets True to capture an NTFF profile
LAST = {}                  # last run's BassKernelResults (for the harness)


def _region_caps(counts, max_c):
    """Water-filled per-class region capacities with upward spill.

    counts: [64, max_c+1] run counts per (core,group) per class.
    Returns caps [max_c+2] and the promoted per-cg counts.
    """
    ncg = counts.shape[0]
    adj = np.zeros((ncg, max_c + 2), dtype=np.int64)
    adj[:, :max_c + 1] = counts
    caps = np.zeros(max_c + 2, dtype=np.int64)
    for c in range(1, max_c + 2):
        col = adj[:, c]
        hi = int(col.max())
        if c <= max_c:
            tgt = int(np.ceil(col.mean() + 1.0 * col.std() + 1))
            caps[c] = min(hi, tgt)
        else:
            caps[c] = hi
        if hi > caps[c] and c <= max_c:
            over = np.maximum(col - caps[c], 0)
            adj[:, c] = col - over
            adj[:, c + 1] += over
    return caps, adj


def _preprocess(edge_src, edge_dst, edge_weight):
    core = (edge_dst // SHARD).astype(np.int64)
    grp = (edge_src // SHARD).astype(np.int64)
    dloc = (edge_dst - core * SHARD).astype(np.int64)
    sloc = (edge_src - grp * SHARD).astype(np.int32)

    key = (core * 8 + grp) * SHARD + dloc
    order = np.argsort(key, kind="stable")
    key_s = key[order]
    sloc_s = sloc[order]
    w_s = edge_weight[order].astype(np.float32)

    uk, start, cnt = np.unique(key_s, return_index=True, return_counts=True)
    run_cg = (uk // SHARD).astype(np.int64)          # core*8+grp
    run_dst = (uk % SHARD).astype(np.int64)
    run_cnt = cnt.astype(np.int64)

    max_c = int(run_cnt.max())
    counts = np.zeros((N_CORES * 8, max_c + 1), dtype=np.int64)
    np.add.at(counts, (run_cg, run_cnt), 1)
    caps, _ = _region_caps(counts, max_c)
    max_c2 = len(caps) - 1
    caps[1] += 1               # shared zero-valued dummy run (last class-1 slot)

    # slot layout: class regions ascending, runs never straddle CHUNK
    # boundaries
    regions = []               # (class, slot_start, run_start, n_runs)
    slot = 0
    run_base = 0
    class_run_off = np.zeros(max_c2 + 2, dtype=np.int64)
    for c in range(1, max_c2 + 1):
        class_run_off[c] = run_base
        n = int(caps[c])
        if n == 0:
            class_run_off[c + 1] = run_base
            continue
        done = 0
        while done < n:
            room = CHUNK - (slot % CHUNK)
            fit = min(n - done, room // c)
            if fit == 0:
                slot += room
                continue
            regions.append((c, slot, run_base + done, fit))
            slot += fit * c
            done += fit
        run_base += n
    class_run_off[max_c2 + 1] = run_base
    total_slots = ((slot + CHUNK - 1) // CHUNK) * CHUNK
    total_runs = run_base
    runs_pad = ((total_runs + 127) // 128) * 128
    assert runs_pad <= 32768 and total_slots // 16 * 16 == total_slots

    run_slot = np.zeros(max(total_runs, 1), dtype=np.int64)
    for (c, s0, r0, n) in regions:
        run_slot[r0:r0 + n] = s0 + np.arange(n, dtype=np.int64) * c
    zero_run = class_run_off[2] - 1     # reserved last class-1 run

    idx_all = np.zeros((N_CORES, 8, total_slots), dtype=np.int16)
    w_all = np.zeros((N_CORES, 8, total_slots), dtype=np.float32)
    perm_all = np.zeros((N_CORES, 8, SHARD_PAD), dtype=np.int16)

    for co in range(N_CORES):
        for g in range(8):
            cg = co * 8 + g
            sel = run_cg == cg
            rc = run_cnt[sel]
            rd = run_dst[sel]
            rstart = start[sel]
            o = np.argsort(rc, kind="stable")
            rc, rd, rstart = rc[o], rd[o], rstart[o]
            # assign runs to regions, promoting overflow upward; never
            # let a run land in the reserved dummy slot of class 1
            ridx = np.zeros(len(rc), dtype=np.int64)
            pos = 0
            used_prev = 0
            for c in range(1, max_c2 + 1):
                k = int((rc == c).sum())
                # runs entering class c: leftover promoted from below + native
                n_here = used_prev + k
                fit = min(n_here, int(caps[c]) - (1 if c == 1 else 0))
                ridx[pos:pos + fit] = class_run_off[c] + np.arange(fit)
                pos += fit
                used_prev = n_here - fit
            assert used_prev == 0, "run overflow beyond top class"
            slots = run_slot[ridx]
            ia = idx_all[co, g]
            wa = w_all[co, g]
            if len(rc):
                tot = int(rc.sum())
                within = np.arange(tot, dtype=np.int64) - np.repeat(
                    np.concatenate([[0], np.cumsum(rc)[:-1]]), rc)
                e_pos = np.repeat(rstart, rc) + within
                s_pos = np.repeat(slots, rc) + within
                ia[s_pos] = sloc_s[e_pos]
                wa[s_pos] = w_s[e_pos]
            pi = np.full(SHARD_PAD, zero_run, dtype=np.int64)
            pi[rd] = ridx
            perm_all[co, g, :] = pi.astype(np.int16)

    def wrap(a):   # [8, S] -> [128, S//16] (i -> partition 16g+i%16, col i//16)
        S = a.shape[1]
        o = np.zeros((128, S // 16), dtype=a.dtype)
        for g in range(8):
            o[16 * g:16 * g + 16, :] = a[g].reshape(-1, 16).T
        return o

    idx_w = np.stack([wrap(idx_all[co]) for co in range(N_CORES)])
    perm_w = np.stack([wrap(perm_all[co]) for co in range(N_CORES)])
    # expanded bf16 weights, chunk-contiguous: [cores, NCH, 128, CHUNK]
    nch = total_slots // CHUNK
    w16 = np.repeat(w_all, 16, axis=1).astype(BF16)          # [cores,128,S]
    w16 = w16.reshape(N_CORES, 128, nch, CHUNK).transpose(0, 2, 1, 3).copy()

    return dict(idx=idx_w, perm=perm_w, w=w16, regions=regions,
                total_slots=total_slots, runs_pad=runs_pad, raw_slots=slot)


def _build(total_slots, regions, runs_pad, raw_slots):
    from concourse import bass, bacc, tile, mybir
    dt = mybir.dt
    AF = mybir.ActivationFunctionType
    nc = bacc.Bacc("TRN2", target_bir_lowering=False, debug=False,
                   num_devices=N_CORES)

    KB = IN_DIM // 128   # 4 k-blocks
    NSLAB = (SHARD_PAD + SLAB - 1) // SLAB
    NCH = total_slots // CHUNK

    fd = nc.dram_tensor("fT", [NSLAB, 128, KB, SLAB], dt.bfloat16, kind="ExternalInput").ap()
    md = nc.dram_tensor("mT", [NSLAB, 128, KB, SLAB], dt.bfloat16, kind="ExternalInput").ap()
    m2d = nc.dram_tensor("m2T", [HID, SHARD_PAD], dt.float32, kind="ExternalInput").ap()
    W1d = nc.dram_tensor("W1", [IN_DIM, HID], dt.bfloat16, kind="ExternalInput").ap()
    b1d = nc.dram_tensor("b1", [HID, 1], dt.float32, kind="ExternalInput").ap()
    W2d = nc.dram_tensor("W2", [HID, OUT], dt.float32, kind="ExternalInput").ap()
    b2d = nc.dram_tensor("b2", [OUT, 1], dt.float32, kind="ExternalInput").ap()
    idxd = nc.dram_tensor("idx", [128, total_slots // 16], dt.int16, kind="ExternalInput").ap()
    permd = nc.dram_tensor("perm", [128, SHARD_PAD // 16], dt.int16, kind="ExternalInput").ap()
    wd = nc.dram_tensor("w", [NCH, 128, CHUNK], dt.bfloat16, kind="ExternalInput").ap()
    seld = nc.dram_tensor("sel", [128, HID], dt.float32, kind="ExternalInput").ap()
    outd = nc.dram_tensor("out", [OUT, SHARD_PAD], dt.float32, kind="ExternalOutput").ap()

    with tile.TileContext(nc) as tc:
        with tc.tile_pool(name="const", bufs=1) as cp, \
             tc.tile_pool(name="big", bufs=1) as bp, \
             tc.tile_pool(name="ps", bufs=2, space="PSUM") as ps, \
             tc.tile_pool(name="dram", bufs=1, space="DRAM") as dp:

            W1 = cp.tile([128, KB, HID], dt.bfloat16)
            b1 = cp.tile([HID, 1], dt.float32)
            W2 = cp.tile([HID, OUT], dt.float32)
            b2 = cp.tile([OUT, 1], dt.float32)
            sel = cp.tile([128, HID], dt.float32)
            idxs = cp.tile([128, total_slots // 16], dt.int16)
            perm = cp.tile([128, SHARD_PAD // 16], dt.int16)
            nc.sync.dma_start(out=W1[:, :, :], in_=W1d.rearrange("(a b) h -> b a h", b=128))
            nc.sync.dma_start(out=b1[:], in_=b1d[:])
            nc.sync.dma_start(out=W2[:], in_=W2d[:])
            nc.sync.dma_start(out=b2[:], in_=b2d[:])
            nc.sync.dma_start(out=sel[:], in_=seld[:])
            nc.sync.dma_start(out=idxs[:], in_=idxd[:])
            nc.sync.dma_start(out=perm[:], in_=permd[:])

            table = bp.tile([128, SHARD_PAD], dt.float32, tag="table")
            st1 = bp.tile([128, runs_pad], dt.float32, tag="st1")

            tableb = bp.tile([128, SHARD_PAD], dt.bfloat16, tag="tableb")
            HA = 6144
            HB = SHARD_PAD - HA
            gi1a = dp.tile([HID, HA], dt.bfloat16, tag="gi1a")
            gi1b = dp.tile([HID, HB], dt.bfloat16, tag="gi1b")
            gi2a = dp.tile([HID, HA], dt.bfloat16, tag="gi2a")
            gi2b = dp.tile([HID, HB], dt.bfloat16, tag="gi2b")
            go1a = dp.tile([128, HA], dt.bfloat16, tag="go1a", addr_space="Shared")
            go1b = dp.tile([128, HB], dt.bfloat16, tag="go1b", addr_space="Shared")
            go2a = dp.tile([128, HA], dt.bfloat16, tag="go2a", addr_space="Shared")
            go2b = dp.tile([128, HB], dt.bfloat16, tag="go2b", addr_space="Shared")

            def allgather_half(gi, go, col0, ncols):
                nc.gpsimd.collective_compute(
                    "AllGather", mybir.AluOpType.bypass,
                    replica_groups=[list(range(N_CORES))],
                    ins=[gi.opt()], outs=[go.opt()])
                # split across 8 DMA queues, then upconvert on DVE
                tw = ncols // 8
                for q in range(8):
                    nc.sync.dma_start(
                        out=tableb[:, col0 + q * tw:col0 + (q + 1) * tw],
                        in_=go[:, q * tw:(q + 1) * tw])
                nc.vector.tensor_copy(out=table[:, col0:col0 + ncols],
                                      in_=tableb[:, col0:col0 + ncols])

            # ---- dense layer 1 (bf16): gi1 = W1^T (f.m) + b1 ------------
            with tc.tile_pool(name="dense", bufs=3) as wkd:
                for s in range(NSLAB):
                    off = s * SLAB
                    jn = min(SLAB, SHARD_PAD - off)
                    f = wkd.tile([128, KB, SLAB], dt.bfloat16, tag="f")
                    m = wkd.tile([128, KB, SLAB], dt.bfloat16, tag="m")
                    nc.sync.dma_start(out=f[:, :, :jn], in_=fd[s, :, :, :jn])
                    nc.sync.dma_start(out=m[:, :, :jn], in_=md[s, :, :, :jn])
                    nc.vector.tensor_tensor(out=f[:, :, :jn], in0=f[:, :, :jn],
                                            in1=m[:, :, :jn], op=mybir.AluOpType.mult)
                    acc = ps.tile([HID, SLAB], dt.float32, tag="acc")
                    for h in range(0, jn, 512):
                        hn = min(512, jn - h)
                        for k in range(KB):
                            nc.tensor.matmul(out=acc[:, h:h + hn],
                                             lhsT=W1[:, k, :],
                                             rhs=f[:, k, h:h + hn],
                                             start=(k == 0), stop=(k == KB - 1))
                    xw = wkd.tile([HID, SLAB], dt.bfloat16, tag="xw")
                    nc.scalar.activation(out=xw[:, :jn], in_=acc[:, :jn],
                                         func=AF.Identity, bias=b1[:], scale=1.0)
                    if off < HA:
                        nc.sync.dma_start(out=gi1a[:, off:off + jn], in_=xw[:, :jn])
                        if off + jn == HA:
                            allgather_half(gi1a, go1a, 0, HA)
                    else:
                        nc.sync.dma_start(out=gi1b[:, off - HA:off - HA + jn],
                                          in_=xw[:, :jn])

            def spmm(wk):
                for ci in range(NCH):
                    cs = ci * CHUNK
                    ni = min(CHUNK, ((raw_slots - cs + 31) // 32) * 32)
                    wch = wk.tile([128, CHUNK], dt.bfloat16, tag="wch")
                    nc.sync.dma_start(out=wch[:, :ni], in_=wd[ci, :, :ni])
                    msg = wk.tile([128, CHUNK], dt.float32, tag="msg")
                    nc.gpsimd.ap_gather(
                        out_ap=msg[:, :ni], in_ap=table[:, :],
                        idxs_ap=idxs[:, cs // 16:cs // 16 + (ni + 15) // 16],
                        channels=128, num_elems=SHARD_PAD, d=1, num_idxs=ni)
                    nc.vector.tensor_tensor(out=msg[:, :ni], in0=msg[:, :ni],
                                            in1=wch[:, :ni], op=mybir.AluOpType.mult)
                    for (c, s0, r0, n) in regions:
                        if s0 < cs or s0 >= cs + CHUNK:
                            continue
                        if c == 1:
                            nc.vector.tensor_copy(out=st1[:, r0:r0 + n],
                                                  in_=msg[:, s0 - cs:s0 - cs + n])
                        else:
                            v = msg[:, s0 - cs:s0 - cs + n * c].rearrange(
                                "p (r c) -> p r c", c=c)
                            nc.vector.tensor_reduce(out=st1[:, r0:r0 + n], in_=v,
                                                    axis=mybir.AxisListType.X,
                                                    op=mybir.AluOpType.add)

            def permloop(wk, post):
                for j in range(0, SHARD_PAD, PERMJ):
                    jn = min(PERMJ, SHARD_PAD - j)
                    al = wk.tile([128, PERMJ], dt.float32, tag="al")
                    nc.gpsimd.ap_gather(
                        out_ap=al[:, :jn], in_ap=st1[:, :],
                        idxs_ap=perm[:, j // 16:(j + jn) // 16],
                        channels=128, num_elems=runs_pad, d=1, num_idxs=jn)
                    acc2 = ps.tile([HID, PERMJ], dt.float32, tag="acc")
                    nc.tensor.matmul(out=acc2[:, :jn], lhsT=sel[:, :], rhs=al[:, :jn],
                                     start=True, stop=True)
                    post(wk, j, jn, acc2)

            # ---- spmm 1 + relu + mask2, allgather 2 ---------------------
            allgather_half(gi1b, go1b, HA, HB)
            with tc.tile_pool(name="spmm1", bufs=3) as wk:
                spmm(wk)

                def post1(wk, j, jn, acc2):
                    xw1 = wk.tile([HID, PERMJ], dt.float32, tag="xw1")
                    nc.scalar.activation(out=xw1[:, :jn], in_=acc2[:, :jn],
                                         func=AF.Relu, scale=1.0)
                    m2c = wk.tile([HID, PERMJ], dt.float32, tag="m2c")
                    nc.sync.dma_start(out=m2c[:, :jn], in_=m2d[:, j:j + jn])
                    xw1b = wk.tile([HID, PERMJ], dt.bfloat16, tag="xw1b")
                    nc.vector.tensor_tensor(out=xw1b[:, :jn], in0=xw1[:, :jn],
                                            in1=m2c[:, :jn], op=mybir.AluOpType.mult)
                    if j < HA:
                        nc.sync.dma_start(out=gi2a[:, j:j + jn], in_=xw1b[:, :jn])
                        if j + jn == HA:
                            allgather_half(gi2a, go2a, 0, HA)
                    else:
                        nc.sync.dma_start(out=gi2b[:, j - HA:j - HA + jn],
                                          in_=xw1b[:, :jn])

                permloop(wk, post1)

            allgather_half(gi2b, go2b, HA, HB)

            # ---- spmm 2, out = W2^T h2 + b2 -----------------------------
            with tc.tile_pool(name="spmm2", bufs=3) as wk:
                spmm(wk)

                def post2(wk, j, jn, acc2):
                    h2s = wk.tile([HID, PERMJ], dt.float32, tag="h2s")
                    nc.scalar.activation(out=h2s[:, :jn], in_=acc2[:, :jn],
                                         func=AF.Identity, scale=1.0)
                    acc3 = ps.tile([OUT, PERMJ], dt.float32, tag="acc3")
                    nc.tensor.matmul(out=acc3[:, :jn], lhsT=W2[:, :], rhs=h2s[:, :jn],
                                     start=True, stop=True)
                    o = wk.tile([OUT, PERMJ], dt.float32, tag="o")
                    nc.scalar.activation(out=o[:, :jn], in_=acc3[:, :jn],
                                         func=AF.Identity, bias=b2[:], scale=1.0)
                    nc.sync.dma_start(out=outd[:, j:j + jn], in_=o[:, :jn])

                permloop(wk, post2)

    nc.compile()
    return nc


def kernel(features, edge_src, edge_dst, edge_weight, mask1, mask2,
           W1, b1, W2, b2):
    from concourse.bass_utils import run_bass_kernel_spmd

    features = np.asarray(features, dtype=np.float32)
    mask1 = np.asarray(mask1, dtype=np.float32)
    mask2 = np.asarray(mask2, dtype=np.float32)
    edge_src = np.asarray(edge_src)
    edge_dst = np.asarray(edge_dst)
    edge_weight = np.asarray(edge_weight, dtype=np.float32)

    pp = _preprocess(edge_src, edge_dst, edge_weight)
    nc = _build(pp["total_slots"], pp["regions"], pp["runs_pad"], pp["raw_slots"])

    sel = np.zeros((128, HID), dtype=np.float32)
    for g in range(8):
        for f in range(HID):
            sel[16 * g + f, f] = 1.0

    KB = IN_DIM // 128
    NSLAB = (SHARD_PAD + SLAB - 1) // SLAB

    def slabify(arr_t):        # [IN_DIM, SHARD_PAD] -> [NSLAB, 128, KB, SLAB]
        o = np.zeros((NSLAB, 128, KB, SLAB), dtype=BF16)
        for s in range(NSLAB):
            jn = min(SLAB, SHARD_PAD - s * SLAB)
            blk = arr_t[:, s * SLAB:s * SLAB + jn]          # [512, jn]
            o[s, :, :, :jn] = blk.reshape(KB, 128, jn).transpose(1, 0, 2)
        return o

    in_maps = []
    for c in range(N_CORES):
        lo, hi = c * SHARD, (c + 1) * SHARD
        fT = np.zeros((IN_DIM, SHARD_PAD), dtype=BF16)
        mT = np.zeros((IN_DIM, SHARD_PAD), dtype=BF16)
        m2T = np.zeros((HID, SHARD_PAD), dtype=np.float32)
        fT[:, :SHARD] = features[lo:hi].T.astype(BF16)
        mT[:, :SHARD] = mask1[lo:hi].T.astype(BF16)
        m2T[:, :SHARD] = mask2[lo:hi].T
        in_maps.append({
            "fT": slabify(fT), "mT": slabify(mT), "m2T": m2T,
            "W1": np.asarray(W1, dtype=np.float32).reshape(IN_DIM, HID).astype(BF16),
            "b1": np.asarray(b1, dtype=np.float32).reshape(HID, 1),
            "W2": np.asarray(W2, dtype=np.float32).reshape(HID, OUT),
            "b2": np.asarray(b2, dtype=np.float32).reshape(OUT, 1),
            "idx": pp["idx"][c], "perm": pp["perm"][c], "w": pp["w"][c],
            "sel": sel,
        })

    res = run_bass_kernel_spmd(nc, in_maps, core_ids=list(range(N_CORES)),
                               trace=TRACE)
    LAST["res"] = res
    out = np.zeros((N_NODES, OUT), dtype=np.float32)
    for c in range(N_CORES):
        out[c * SHARD:(c + 1) * SHARD] = res.results[c]["out"][:, :SHARD].T
    return out


# revision 14
# speedup vs baseline: 1.0390x; 1.0097x over previous
"""2-layer GCN on 8 Trainium2 NeuronCores via Bass/Tile.

Sharding: nodes row-sharded across the 8 cores (12500 each, padded to
12544); edges sharded by destination core and grouped by source shard
(the 8 GPSIMD Q7 cores).  Dense transforms run feats-on-partitions in
bf16 with the weights stationary; the 16-dim xw is all-gathered into a
per-core fp32 SBUF table [128 partitions = 8 src shards x 16 feats,
12544 nodes]; messages are gathered with gpsimd.ap_gather (the kernel's
hard bottleneck at ~27.5 ns/idx/core), weighted with a streamed bf16
expanded weight tile and segment-reduced on the vector engine
((dst,group) runs class-sorted by length with water-filled region
capacities + upward spill to cut padding), permuted back to dst order
with a second ap_gather and combined across groups with one PE matmul
contracting the partition axis.  W2 is applied after the second spmm
(it commutes with the segment sum).
"""
import sys

for _p in ("/opt/trn_rl_repo",):
    if _p not in sys.path:
        sys.path.insert(0, _p)

import numpy as np
import ml_dtypes

BF16 = ml_dtypes.bfloat16

N_CORES = 8
N_NODES = 100000
SHARD = 12500
SHARD_PAD = 12544          # 98*128
IN_DIM = 512
HID = 16
OUT = 7
CHUNK = 2048               # gather slots per ap_gather call (per group)
SLAB = 1024                # dense-phase node slab
PERMJ = 512                # perm-gather block

TRACE = False              # test harness sets True to capture an NTFF profile
LAST = {}                  # last run's BassKernelResults (for the harness)


def _region_caps(counts, max_c):
    """Water-filled per-class region capacities with upward spill.

    counts: [64, max_c+1] run counts per (core,group) per class.
    Returns caps [max_c+2] and the promoted per-cg counts.
    """
    ncg = counts.shape[0]
    adj = np.zeros((ncg, max_c + 2), dtype=np.int64)
    adj[:, :max_c + 1] = counts
    caps = np.zeros(max_c + 2, dtype=np.int64)
    for c in range(1, max_c + 2):
        col = adj[:, c]
        hi = int(col.max())
        if c <= max_c:
            tgt = int(np.ceil(col.mean() + 1.0 * col.std() + 1))
            caps[c] = min(hi, tgt)
        else:
            caps[c] = hi
        if hi > caps[c] and c <= max_c:
            over = np.maximum(col - caps[c], 0)
            adj[:, c] = col - over
            adj[:, c + 1] += over
    return caps, adj


def _preprocess(edge_src, edge_dst, edge_weight):
    core = (edge_dst // SHARD).astype(np.int64)
    grp = (edge_src // SHARD).astype(np.int64)
    dloc = (edge_dst - core * SHARD).astype(np.int64)
    sloc = (edge_src - grp * SHARD).astype(np.int32)

    key = (core * 8 + grp) * SHARD + dloc
    order = np.argsort(key, kind="stable")
    key_s = key[order]
    sloc_s = sloc[order]
    w_s = edge_weight[order].astype(np.float32)

    uk, start, cnt = np.unique(key_s, return_index=True, return_counts=True)
    run_cg = (uk // SHARD).astype(np.int64)          # core*8+grp
    run_dst = (uk % SHARD).astype(np.int64)
    run_cnt = cnt.astype(np.int64)

    max_c = int(run_cnt.max())
    counts = np.zeros((N_CORES * 8, max_c + 1), dtype=np.int64)
    np.add.at(counts, (run_cg, run_cnt), 1)
    caps, _ = _region_caps(counts, max_c)
    max_c2 = len(caps) - 1
    caps[1] += 1               # shared zero-valued dummy run (last class-1 slot)

    # slot layout: class regions ascending, runs never straddle CHUNK
    # boundaries
    regions = []               # (class, slot_start, run_start, n_runs)
    slot = 0
    run_base = 0
    class_run_off = np.zeros(max_c2 + 2, dtype=np.int64)
    for c in range(1, max_c2 + 1):
        class_run_off[c] = run_base
        n = int(caps[c])
        if n == 0:
            class_run_off[c + 1] = run_base
            continue
        done = 0
        while done < n:
            room = CHUNK - (slot % CHUNK)
            fit = min(n - done, room // c)
            if fit == 0:
                slot += room
                continue
            regions.append((c, slot, run_base + done, fit))
            slot += fit * c
            done += fit
        run_base += n
    class_run_off[max_c2 + 1] = run_base
    total_slots = ((slot + CHUNK - 1) // CHUNK) * CHUNK
    total_runs = run_base
    runs_pad = ((total_runs + 127) // 128) * 128
    assert runs_pad <= 32768 and total_slots // 16 * 16 == total_slots

    run_slot = np.zeros(max(total_runs, 1), dtype=np.int64)
    for (c, s0, r0, n) in regions:
        run_slot[r0:r0 + n] = s0 + np.arange(n, dtype=np.int64) * c
    zero_run = class_run_off[2] - 1     # reserved last class-1 run

    idx_all = np.zeros((N_CORES, 8, total_slots), dtype=np.int16)
    w_all = np.zeros((N_CORES, 8, total_slots), dtype=np.float32)
    perm_all = np.zeros((N_CORES, 8, SHARD_PAD), dtype=np.int16)

    for co in range(N_CORES):
        for g in range(8):
            cg = co * 8 + g
            sel = run_cg == cg
            rc = run_cnt[sel]
            rd = run_dst[sel]
            rstart = start[sel]
            o = np.argsort(rc, kind="stable")
            rc, rd, rstart = rc[o], rd[o], rstart[o]
            # assign runs to regions, promoting overflow upward; never
            # let a run land in the reserved dummy slot of class 1
            ridx = np.zeros(len(rc), dtype=np.int64)
            pos = 0
            used_prev = 0
            for c in range(1, max_c2 + 1):
                k = int((rc == c).sum())
                # runs entering class c: leftover promoted from below + native
                n_here = used_prev + k
                fit = min(n_here, int(caps[c]) - (1 if c == 1 else 0))
                ridx[pos:pos + fit] = class_run_off[c] + np.arange(fit)
                pos += fit
                used_prev = n_here - fit
            assert used_prev == 0, "run overflow beyond top class"
            slots = run_slot[ridx]
            ia = idx_all[co, g]
            wa = w_all[co, g]
            if len(rc):
                tot = int(rc.sum())
                within = np.arange(tot, dtype=np.int64) - np.repeat(
                    np.concatenate([[0], np.cumsum(rc)[:-1]]), rc)
                e_pos = np.repeat(rstart, rc) + within
                s_pos = np.repeat(slots, rc) + within
                ia[s_pos] = sloc_s[e_pos]
                wa[s_pos] = w_s[e_pos]
            pi = np.full(SHARD_PAD, zero_run, dtype=np.int64)
            pi[rd] = ridx
            perm_all[co, g, :] = pi.astype(np.int16)

    def wrap(a):   # [8, S] -> [128, S//16] (i -> partition 16g+i%16, col i//16)
        S = a.shape[1]
        o = np.zeros((128, S // 16), dtype=a.dtype)
        for g in range(8):
            o[16 * g:16 * g + 16, :] = a[g].reshape(-1, 16).T
        return o

    idx_w = np.stack([wrap(idx_all[co]) for co in range(N_CORES)])
    perm_w = np.stack([wrap(perm_all[co]) for co in range(N_CORES)])
    # expanded bf16 weights, chunk-contiguous: [cores, NCH, 128, CHUNK]
    nch = total_slots // CHUNK
    w16 = np.repeat(w_all, 16, axis=1).astype(BF16)          # [cores,128,S]
    w16 = w16.reshape(N_CORES, 128, nch, CHUNK).transpose(0, 2, 1, 3).copy()

    return dict(idx=idx_w, perm=perm_w, w=w16, regions=regions,
                total_slots=total_slots, runs_pad=runs_pad, raw_slots=slot)


def _build(total_slots, regions, runs_pad, raw_slots):
    from concourse import bass, bacc, tile, mybir
    dt = mybir.dt
    AF = mybir.ActivationFunctionType
    nc = bacc.Bacc("TRN2", target_bir_lowering=False, debug=False,
                   num_devices=N_CORES)

    KB = IN_DIM // 128   # 4 k-blocks
    NSLAB = (SHARD_PAD + SLAB - 1) // SLAB
    NCH = total_slots // CHUNK

    fd = nc.dram_tensor("fT", [NSLAB, 128, KB, SLAB], dt.bfloat16, kind="ExternalInput").ap()
    md = nc.dram_tensor("mT", [NSLAB, 128, KB, SLAB], dt.bfloat16, kind="ExternalInput").ap()
    m2d = nc.dram_tensor("m2T", [HID, SHARD_PAD], dt.float32, kind="ExternalInput").ap()
    W1d = nc.dram_tensor("W1", [IN_DIM, HID], dt.bfloat16, kind="ExternalInput").ap()
    b1d = nc.dram_tensor("b1", [HID, 1], dt.float32, kind="ExternalInput").ap()
    W2d = nc.dram_tensor("W2", [HID, OUT], dt.float32, kind="ExternalInput").ap()
    b2d = nc.dram_tensor("b2", [OUT, 1], dt.float32, kind="ExternalInput").ap()
    idxd = nc.dram_tensor("idx", [128, total_slots // 16], dt.int16, kind="ExternalInput").ap()
    permd = nc.dram_tensor("perm", [128, SHARD_PAD // 16], dt.int16, kind="ExternalInput").ap()
    wd = nc.dram_tensor("w", [NCH, 128, CHUNK], dt.bfloat16, kind="ExternalInput").ap()
    seld = nc.dram_tensor("sel", [128, HID], dt.float32, kind="ExternalInput").ap()
    outd = nc.dram_tensor("out", [OUT, SHARD_PAD], dt.float32, kind="ExternalOutput").ap()

    with tile.TileContext(nc) as tc:
        with tc.tile_pool(name="const", bufs=1) as cp, \
             tc.tile_pool(name="big", bufs=1) as bp, \
             tc.tile_pool(name="ps", bufs=2, space="PSUM") as ps, \
             tc.tile_pool(name="dram", bufs=1, space="DRAM") as dp:

            W1 = cp.tile([128, KB, HID], dt.bfloat16)
            b1 = cp.tile([HID, 1], dt.float32)
            W2 = cp.tile([HID, OUT], dt.float32)
            b2 = cp.tile([OUT, 1], dt.float32)
            sel = cp.tile([128, HID], dt.float32)
            idxs = cp.tile([128, total_slots // 16], dt.int16)
            perm = cp.tile([128, SHARD_PAD // 16], dt.int16)
            nc.sync.dma_start(out=W1[:, :, :], in_=W1d.rearrange("(a b) h -> b a h", b=128))
            nc.sync.dma_start(out=b1[:], in_=b1d[:])
            nc.sync.dma_start(out=W2[:], in_=W2d[:])
            nc.sync.dma_start(out=b2[:], in_=b2d[:])
            nc.sync.dma_start(out=sel[:], in_=seld[:])
            nc.sync.dma_start(out=idxs[:], in_=idxd[:])
            nc.sync.dma_start(out=perm[:], in_=permd[:])

            table = bp.tile([128, SHARD_PAD], dt.float32, tag="table")
            st1 = bp.tile([128, runs_pad], dt.float32, tag="st1")

            tableb = bp.tile([128, SHARD_PAD], dt.bfloat16, tag="tableb")
            HA = 6144
            HB = SHARD_PAD - HA
            gi1a = dp.tile([HID, HA], dt.bfloat16, tag="gi1a")
            gi1b = dp.tile([HID, HB], dt.bfloat16, tag="gi1b")
            gi2a = dp.tile([HID, HA], dt.bfloat16, tag="gi2a")
            gi2b = dp.tile([HID, HB], dt.bfloat16, tag="gi2b")
            go1a = dp.tile([128, HA], dt.bfloat16, tag="go1a", addr_space="Shared")
            go1b = dp.tile([128, HB], dt.bfloat16, tag="go1b", addr_space="Shared")
            go2a = dp.tile([128, HA], dt.bfloat16, tag="go2a", addr_space="Shared")
            go2b = dp.tile([128, HB], dt.bfloat16, tag="go2b", addr_space="Shared")

            def allgather_half(gi, go, col0, ncols):
                nc.gpsimd.collective_compute(
                    "AllGather", mybir.AluOpType.bypass,
                    replica_groups=[list(range(N_CORES))],
                    ins=[gi.opt()], outs=[go.opt()])
                # split across 8 DMA queues, then upconvert on DVE
                tw = ncols // 8
                for q in range(8):
                    nc.sync.dma_start(
                        out=tableb[:, col0 + q * tw:col0 + (q + 1) * tw],
                        in_=go[:, q * tw:(q + 1) * tw])
                nc.vector.tensor_copy(out=table[:, col0:col0 + ncols],
                                      in_=tableb[:, col0:col0 + ncols])

            # ---- dense layer 1 (bf16): gi1 = W1^T (f.m) + b1 ------------
            with tc.tile_pool(name="dense", bufs=3) as wkd:
                for s in range(NSLAB):
                    off = s * SLAB
                    jn = min(SLAB, SHARD_PAD - off)
                    f = wkd.tile([128, KB, SLAB], dt.bfloat16, tag="f")
                    m = wkd.tile([128, KB, SLAB], dt.bfloat16, tag="m")
                    nc.sync.dma_start(out=f[:, :, :jn], in_=fd[s, :, :, :jn])
                    nc.sync.dma_start(out=m[:, :, :jn], in_=md[s, :, :, :jn])
                    nc.vector.tensor_tensor(out=f[:, :, :jn], in0=f[:, :, :jn],
                                            in1=m[:, :, :jn], op=mybir.AluOpType.mult)
                    acc = ps.tile([HID, SLAB], dt.float32, tag="acc")
                    for h in range(0, jn, 512):
                        hn = min(512, jn - h)
                        for k in range(KB):
                            nc.tensor.matmul(out=acc[:, h:h + hn],
                                             lhsT=W1[:, k, :],
                                             rhs=f[:, k, h:h + hn],
                                             start=(k == 0), stop=(k == KB - 1))
                    xw = wkd.tile([HID, SLAB], dt.bfloat16, tag="xw")
                    nc.scalar.activation(out=xw[:, :jn], in_=acc[:, :jn],
                                         func=AF.Identity, bias=b1[:], scale=1.0)
                    if off < HA:
                        nc.sync.dma_start(out=gi1a[:, off:off + jn], in_=xw[:, :jn])
                        if off + jn == HA:
                            allgather_half(gi1a, go1a, 0, HA)
                    else:
                        nc.sync.dma_start(out=gi1b[:, off - HA:off - HA + jn],
                                          in_=xw[:, :jn])

            def spmm(wk):
                for ci in range(NCH):
                    cs = ci * CHUNK
                    ni = min(CHUNK, ((raw_slots - cs + 31) // 32) * 32)
                    wch = wk.tile([128, CHUNK], dt.bfloat16, tag="wch")
                    nc.sync.dma_start(out=wch[:, :ni], in_=wd[ci, :, :ni])
                    msg = wk.tile([128, CHUNK], dt.float32, tag="msg", bufs=4)
                    nc.gpsimd.ap_gather(
                        out_ap=msg[:, :ni], in_ap=table[:, :],
                        idxs_ap=idxs[:, cs // 16:cs // 16 + (ni + 15) // 16],
                        channels=128, num_elems=SHARD_PAD, d=1, num_idxs=ni)
                    nc.vector.tensor_tensor(out=msg[:, :ni], in0=msg[:, :ni],
                                            in1=wch[:, :ni], op=mybir.AluOpType.mult)
                    for (c, s0, r0, n) in regions:
                        if s0 < cs or s0 >= cs + CHUNK:
                            continue
                        if c == 1:
                            nc.vector.tensor_copy(out=st1[:, r0:r0 + n],
                                                  in_=msg[:, s0 - cs:s0 - cs + n])
                        else:
                            v = msg[:, s0 - cs:s0 - cs + n * c].rearrange(
                                "p (r c) -> p r c", c=c)
                            nc.vector.tensor_reduce(out=st1[:, r0:r0 + n], in_=v,
                                                    axis=mybir.AxisListType.X,
                                                    op=mybir.AluOpType.add)

            def permloop(wk, post):
                for j in range(0, SHARD_PAD, PERMJ):
                    jn = min(PERMJ, SHARD_PAD - j)
                    al = wk.tile([128, PERMJ], dt.float32, tag="al")
                    nc.gpsimd.ap_gather(
                        out_ap=al[:, :jn], in_ap=st1[:, :],
                        idxs_ap=perm[:, j // 16:(j + jn) // 16],
                        channels=128, num_elems=runs_pad, d=1, num_idxs=jn)
                    acc2 = ps.tile([HID, PERMJ], dt.float32, tag="acc")
                    nc.tensor.matmul(out=acc2[:, :jn], lhsT=sel[:, :], rhs=al[:, :jn],
                                     start=True, stop=True)
                    post(wk, j, jn, acc2)

            # ---- spmm 1 + relu + mask2, allgather 2 ---------------------
            allgather_half(gi1b, go1b, HA, HB)
            with tc.tile_pool(name="spmm1", bufs=3) as wk:
                spmm(wk)

                def post1(wk, j, jn, acc2):
                    xw1 = wk.tile([HID, PERMJ], dt.float32, tag="xw1")
                    nc.scalar.activation(out=xw1[:, :jn], in_=acc2[:, :jn],
                                         func=AF.Relu, scale=1.0)
                    m2c = wk.tile([HID, PERMJ], dt.float32, tag="m2c")
                    nc.sync.dma_start(out=m2c[:, :jn], in_=m2d[:, j:j + jn])
                    xw1b = wk.tile([HID, PERMJ], dt.bfloat16, tag="xw1b")
                    nc.vector.tensor_tensor(out=xw1b[:, :jn], in0=xw1[:, :jn],
                                            in1=m2c[:, :jn], op=mybir.AluOpType.mult)
                    if j < HA:
                        nc.sync.dma_start(out=gi2a[:, j:j + jn], in_=xw1b[:, :jn])
                        if j + jn == HA:
                            allgather_half(gi2a, go2a, 0, HA)
                    else:
                        nc.sync.dma_start(out=gi2b[:, j - HA:j - HA + jn],
                                          in_=xw1b[:, :jn])

                permloop(wk, post1)

            allgather_half(gi2b, go2b, HA, HB)

            # ---- spmm 2, out = W2^T h2 + b2 -----------------------------
            with tc.tile_pool(name="spmm2", bufs=3) as wk:
                spmm(wk)

                def post2(wk, j, jn, acc2):
                    h2s = wk.tile([HID, PERMJ], dt.float32, tag="h2s")
                    nc.scalar.activation(out=h2s[:, :jn], in_=acc2[:, :jn],
                                         func=AF.Identity, scale=1.0)
                    acc3 = ps.tile([OUT, PERMJ], dt.float32, tag="acc3")
                    nc.tensor.matmul(out=acc3[:, :jn], lhsT=W2[:, :], rhs=h2s[:, :jn],
                                     start=True, stop=True)
                    o = wk.tile([OUT, PERMJ], dt.float32, tag="o")
                    nc.scalar.activation(out=o[:, :jn], in_=acc3[:, :jn],
                                         func=AF.Identity, bias=b2[:], scale=1.0)
                    nc.sync.dma_start(out=outd[:, j:j + jn], in_=o[:, :jn])

                permloop(wk, post2)

    nc.compile()
    return nc


def kernel(features, edge_src, edge_dst, edge_weight, mask1, mask2,
           W1, b1, W2, b2):
    from concourse.bass_utils import run_bass_kernel_spmd

    features = np.asarray(features, dtype=np.float32)
    mask1 = np.asarray(mask1, dtype=np.float32)
    mask2 = np.asarray(mask2, dtype=np.float32)
    edge_src = np.asarray(edge_src)
    edge_dst = np.asarray(edge_dst)
    edge_weight = np.asarray(edge_weight, dtype=np.float32)

    pp = _preprocess(edge_src, edge_dst, edge_weight)
    nc = _build(pp["total_slots"], pp["regions"], pp["runs_pad"], pp["raw_slots"])

    sel = np.zeros((128, HID), dtype=np.float32)
    for g in range(8):
        for f in range(HID):
            sel[16 * g + f, f] = 1.0

    KB = IN_DIM // 128
    NSLAB = (SHARD_PAD + SLAB - 1) // SLAB

    def slabify(arr_t):        # [IN_DIM, SHARD_PAD] -> [NSLAB, 128, KB, SLAB]
        o = np.zeros((NSLAB, 128, KB, SLAB), dtype=BF16)
        for s in range(NSLAB):
            jn = min(SLAB, SHARD_PAD - s * SLAB)
            blk = arr_t[:, s * SLAB:s * SLAB + jn]          # [512, jn]
            o[s, :, :, :jn] = blk.reshape(KB, 128, jn).transpose(1, 0, 2)
        return o

    in_maps = []
    for c in range(N_CORES):
        lo, hi = c * SHARD, (c + 1) * SHARD
        fT = np.zeros((IN_DIM, SHARD_PAD), dtype=BF16)
        mT = np.zeros((IN_DIM, SHARD_PAD), dtype=BF16)
        m2T = np.zeros((HID, SHARD_PAD), dtype=np.float32)
        fT[:, :SHARD] = features[lo:hi].T.astype(BF16)
        mT[:, :SHARD] = mask1[lo:hi].T.astype(BF16)
        m2T[:, :SHARD] = mask2[lo:hi].T
        in_maps.append({
            "fT": slabify(fT), "mT": slabify(mT), "m2T": m2T,
            "W1": np.asarray(W1, dtype=np.float32).reshape(IN_DIM, HID).astype(BF16),
            "b1": np.asarray(b1, dtype=np.float32).reshape(HID, 1),
            "W2": np.asarray(W2, dtype=np.float32).reshape(HID, OUT),
            "b2": np.asarray(b2, dtype=np.float32).reshape(OUT, 1),
            "idx": pp["idx"][c], "perm": pp["perm"][c], "w": pp["w"][c],
            "sel": sel,
        })

    res = run_bass_kernel_spmd(nc, in_maps, core_ids=list(range(N_CORES)),
                               trace=TRACE)
    LAST["res"] = res
    out = np.zeros((N_NODES, OUT), dtype=np.float32)
    for c in range(N_CORES):
        out[c * SHARD:(c + 1) * SHARD] = res.results[c]["out"][:, :SHARD].T
    return out
